# revision 1
# baseline (speedup 1.0000x reference)
"""AttentionBlock (GroupNorm32 + 8-head self-attention + proj + residual) on
8 Trainium2 NeuronCores, data-parallel over the batch (B=8 -> 1 element/core).

kernel(**inputs) takes the FULL unsharded inputs (numpy) and returns the FULL
output [8, 512, 32, 32].

Per-core device program (C=512 ch, N=1024 px, 8 heads, hd=64):
  xn  = (x - mean_g) * rsqrt(var_g + eps)        GroupNorm, gamma/beta folded
                                                 into qkv weights on the host
  q,k = Wqk xn + b   (q pre-scaled by 1/8)       [ch-on-partitions layout]
  vT  = xn^T Wv^T    (+ ones column)             [px-on-partitions layout]
  P   = exp(k_h^T q_h)  (|logits| < 7 -> no max subtraction needed)
  o,Z = vT_h^T P        (ones column of vT yields the softmax denominator Z)
  ao  = o / Z
  y   = x + projW ao + proj_b'                   (v-bias folded through proj)

All matmuls run as float32r (full-rate fp32 mode of the PE).
"""

import sys

if "/opt/trn_rl_repo" not in sys.path:
    sys.path.insert(0, "/opt/trn_rl_repo")

import numpy as np

import concourse.bass as bass
import concourse.tile as tile
from concourse import bacc, mybir
from concourse.alu_op_type import AluOpType
from concourse.bass_utils import run_bass_kernel_spmd

F32 = mybir.dt.float32
F32R = mybir.dt.float32r
AF = mybir.ActivationFunctionType

C = 512          # channels
N = 1024         # pixels (32x32)
NH = 8           # heads
HD = 64          # head dim
GS = 16          # channels per groupnorm group
EPS = 1e-5
CT = C // 128    # channel tiles
JT = N // 128    # pixel tiles
IC = N // 512    # moving chunks
NCORES = 8


def _host_prep(x, gn_gamma, gn_beta, qkv_w, qkv_b, proj_w, proj_b):
    f = np.float32
    gamma = np.asarray(gn_gamma, f)
    beta = np.asarray(gn_beta, f)
    qkv_w = np.asarray(qkv_w, f)
    qkv_b = np.asarray(qkv_b, f)
    proj_w = np.asarray(proj_w, f)
    proj_b = np.asarray(proj_b, f)
    scale = f(HD) ** f(-0.5)

    Wq, Wk, Wv = qkv_w[0:C], qkv_w[C:2 * C], qkv_w[2 * C:3 * C]
    bq = (qkv_b[0:C] + Wq @ beta) * scale
    bk = qkv_b[C:2 * C] + Wk @ beta
    bv = qkv_b[2 * C:3 * C] + Wv @ beta
    Wq = Wq * gamma[None, :] * scale
    Wk = Wk * gamma[None, :]
    Wv = Wv * gamma[None, :]

    A = np.zeros((128, 8), f)
    A[np.arange(128), np.arange(128) // GS] = f(1.0 / GS)
    E = np.zeros((8, 128), f)
    E[np.arange(128) // GS, np.arange(128)] = f(1.0)

    weights = {
        "wqkT": np.ascontiguousarray(np.concatenate([Wq, Wk], 0).T),
        "wvT": np.ascontiguousarray(Wv.T),
        "qkb": np.concatenate([bq, bk]),
        "pwT": np.ascontiguousarray(proj_w.T),
        "pb": proj_b + proj_w @ bv,
        "gA": A, "gE": E,
    }
    xs = [np.ascontiguousarray(np.asarray(x[b], f).reshape(C, N))
          for b in range(x.shape[0])]
    return weights, xs


def _declare_io(nc):
    io = {}
    io["x"] = nc.dram_tensor("x", [C, N], F32, kind="ExternalInput")
    io["wqkT"] = nc.dram_tensor("wqkT", [C, 2 * C], F32, kind="ExternalInput")
    io["wvT"] = nc.dram_tensor("wvT", [C, C], F32, kind="ExternalInput")
    io["qkb"] = nc.dram_tensor("qkb", [2 * C], F32, kind="ExternalInput")
    io["pwT"] = nc.dram_tensor("pwT", [C, C], F32, kind="ExternalInput")
    io["pb"] = nc.dram_tensor("pb", [C], F32, kind="ExternalInput")
    io["gA"] = nc.dram_tensor("gA", [128, 8], F32, kind="ExternalInput")
    io["gE"] = nc.dram_tensor("gE", [8, 128], F32, kind="ExternalInput")
    io["out"] = nc.dram_tensor("out", [C, N], F32, kind="ExternalOutput")
    return io


def _build(nc, io, mm_dtype=F32R, p_bufs=10):
    def mm(ap):
        return ap.bitcast(mm_dtype) if mm_dtype != F32 else ap

    with tile.TileContext(nc) as tc:
        with (
            tc.tile_pool(name="const", bufs=1) as const,
            tc.tile_pool(name="big", bufs=1) as big,
            tc.tile_pool(name="pp", bufs=p_bufs) as ppool,
            tc.tile_pool(name="sm", bufs=4) as sm,
            tc.tile_pool(name="zbp", bufs=4) as zbp,
            tc.tile_pool(name="zr", bufs=2) as zrp,
            tc.tile_pool(name="zdp", bufs=4, space="DRAM") as zdp,
            tc.tile_pool(name="psA", bufs=3, space=bass.MemorySpace.PSUM) as psA,
            tc.tile_pool(name="psB", bufs=2, space=bass.MemorySpace.PSUM) as psB,
        ):
            # ---- weight / input loads ----------------------------------
            # x on the sync queue first (GroupNorm needs it); weights spread
            # over other engines' DMA queues so they don't delay x.
            wqk_sb, wv_sb, pw_sb, x_sb = [], [], [], []
            for t in range(CT):
                w1 = const.tile([128, 2 * C], F32, tag=f"wqk{t}", name=f"wqk{t}")
                nc.sync.dma_start(out=mm(w1[:]),
                                  in_=mm(io["wqkT"][128 * t:128 * (t + 1), :]))
                wqk_sb.append(w1)
                w2 = const.tile([128, C], F32, tag=f"wv{t}", name=f"wv{t}")
                nc.sync.dma_start(out=mm(w2[:]),
                                  in_=mm(io["wvT"][128 * t:128 * (t + 1), :]))
                wv_sb.append(w2)
                w3 = const.tile([128, C], F32, tag=f"pw{t}", name=f"pw{t}")
                nc.sync.dma_start(out=mm(w3[:]),
                                  in_=mm(io["pwT"][128 * t:128 * (t + 1), :]))
                pw_sb.append(w3)
                xt = big.tile([128, N], F32, tag=f"x{t}", name=f"x{t}")
                nc.sync.dma_start(out=xt[:], in_=io["x"][128 * t:128 * (t + 1), :])
                x_sb.append(xt)

            qkb_sb = const.tile([128, 8], F32, tag="qkb", name="qkb")
            nc.sync.dma_start(out=qkb_sb[:],
                              in_=io["qkb"][:].rearrange("(t p) -> p t", p=128))
            pb_sb = const.tile([128, 4], F32, tag="pb", name="pb")
            nc.sync.dma_start(out=pb_sb[:],
                              in_=io["pb"][:].rearrange("(t p) -> p t", p=128))
            A_sb = const.tile([128, 8], F32, tag="gA", name="gA")
            nc.sync.dma_start(out=A_sb[:], in_=io["gA"][:])
            E_sb = const.tile([8, 128], F32, tag="gE", name="gE")
            nc.sync.dma_start(out=E_sb[:], in_=io["gE"][:])
            eps_sb = const.tile([128, 1], F32, tag="eps", name="eps")
            nc.vector.memset(eps_sb[:], EPS)
            ones_sb = const.tile([128, 1], F32, tag="ones", name="ones")
            nc.vector.memset(ones_sb[:], 1.0)

            # ---- GroupNorm ---------------------------------------------
            # per-channel mean / E[x^2] via bn_stats (free-dim reduction) ...
            stats_all = sm.tile([128, 8], F32, tag="stats_all", name="stats_all")
            for t in range(CT):
                st = sm.tile([128, 2, 6], F32, tag="bnst", name="bnst")
                nc.vector.bn_stats(out=st[:, 0, :], in_=x_sb[t][:, 0:512])
                nc.vector.bn_stats(out=st[:, 1, :], in_=x_sb[t][:, 512:1024])
                mv = sm.tile([128, 2], F32, tag="bnmv", name="bnmv")
                nc.vector.bn_aggr(out=mv[:], in_=st[:])
                nc.vector.tensor_copy(out=stats_all[:, 2 * t:2 * t + 1], in_=mv[:, 0:1])
                nc.vector.scalar_tensor_tensor(
                    out=stats_all[:, 2 * t + 1:2 * t + 2],
                    in0=mv[:, 0:1], scalar=mv[:, 0:1], in1=mv[:, 1:2],
                    op0=AluOpType.mult, op1=AluOpType.add)

            # ... then group-aggregate across partitions with a tiny matmul
            ps_g = psB.tile([8, 8], F32, tag="ps", name="ps")
            nc.tensor.matmul(ps_g[:], lhsT=A_sb[:], rhs=stats_all[:],
                             start=True, stop=True)
            gs = sm.tile([8, 8], F32, tag="gs", name="gs")
            nc.vector.tensor_copy(out=gs[:], in_=ps_g[:])
            gsr = gs[:].rearrange("p (t s) -> p s t", s=2)
            gmean, gex2 = gsr[:, 0, :], gsr[:, 1, :]
            tmp = sm.tile([8, 2, 4], F32, tag="gtmp", name="gtmp")
            nc.vector.tensor_tensor(out=tmp[:, 0, :], in0=gmean, in1=gmean,
                                    op=AluOpType.mult)
            nc.vector.tensor_tensor(out=tmp[:, 1, :], in0=gex2, in1=tmp[:, 0, :],
                                    op=AluOpType.subtract)
            # rstd = exp(-0.5*ln(var+eps)): Ln/Exp share one ACT table set
            lnv = sm.tile([8, 4], F32, tag="lnv", name="lnv")
            nc.scalar.activation(out=lnv[:], in_=tmp[:, 1, :], func=AF.Ln,
                                 bias=eps_sb[0:8, :])
            gm = sm.tile([8, 2, 4], F32, tag="gm", name="gm")
            nc.scalar.activation(out=gm[:, 0, :], in_=lnv[:], func=AF.Exp, scale=-0.5)
            nc.vector.tensor_copy(out=gm[:, 1, :], in_=gmean)

            # expand group stats back to channels (tiny matmul with E)
            ps_e = psB.tile([128, 8], F32, tag="ps", name="ps")
            nc.tensor.matmul(ps_e[:], lhsT=E_sb[:],
                             rhs=gm[:].rearrange("p s t -> p (s t)"),
                             start=True, stop=True)
            ab = sm.tile([128, 8], F32, tag="ab", name="ab")
            nc.vector.tensor_copy(out=ab[:], in_=ps_e[:])
            bvec = sm.tile([128, 4], F32, tag="bvec", name="bvec")
            nc.vector.tensor_tensor(out=bvec[:], in0=ab[:, 4:8], in1=ab[:, 0:4],
                                    op=AluOpType.mult)
            nc.vector.tensor_scalar_mul(bvec[:], bvec[:], -1.0)

            xn_sb = []
            for t in range(CT):
                xnt = big.tile([128, N], F32, tag=f"xn{t}", name=f"xn{t}")
                nc.scalar.activation(out=mm(xnt[:]), in_=x_sb[t][:],
                                     func=AF.Identity,
                                     scale=ab[:, t:t + 1], bias=bvec[:, t:t + 1])
                xn_sb.append(xnt)

            # ---- q/k/vT projections ------------------------------------
            q_sb = [big.tile([128, N], F32, tag=f"q{t}", name=f"q{t}")
                    for t in range(CT)]
            k_sb = [big.tile([128, N], F32, tag=f"k{t}", name=f"k{t}")
                    for t in range(CT)]
            for ot in range(8):          # 0..3 -> q tiles, 4..7 -> k tiles
                dst = q_sb[ot] if ot < 4 else k_sb[ot - 4]
                for ic in range(IC):
                    ps = psB.tile([128, 512], F32, tag="ps", name="ps")
                    for kt in range(CT):
                        nc.tensor.matmul(
                            ps[:],
                            lhsT=mm(wqk_sb[kt][:, 128 * ot:128 * (ot + 1)]),
                            rhs=mm(xn_sb[kt][:, 512 * ic:512 * (ic + 1)]),
                            start=(kt == 0), stop=(kt == CT - 1))
                    nc.vector.tensor_scalar_add(
                        mm(dst[:, 512 * ic:512 * (ic + 1)]), ps[:],
                        qkb_sb[:, ot:ot + 1])

            vT_sb = []
            for jt in range(JT):
                vt = big.tile([128, NH, HD + 1], F32, tag=f"vT{jt}", name=f"vT{jt}")
                psv = psB.tile([128, 512], F32, tag="ps", name="ps")
                for kt in range(CT):
                    nc.tensor.matmul(
                        psv[:],
                        lhsT=mm(xn_sb[kt][:, 128 * jt:128 * (jt + 1)]),
                        rhs=mm(wv_sb[kt][:]),
                        start=(kt == 0), stop=(kt == CT - 1))
                nc.vector.tensor_copy(
                    out=mm(vt[:, :, 0:HD]),
                    in_=psv[:].rearrange("p (h c) -> p h c", h=NH))
                nc.vector.tensor_copy(
                    out=mm(vt[:, :, HD:HD + 1]),
                    in_=ones_sb[:].to_broadcast((128, NH, 1)))
                vT_sb.append(vt)

            # ---- attention ---------------------------------------------
            ao_sb = [big.tile([128, N], F32, tag=f"ao{t}", name=f"ao{t}")
                     for t in range(CT)]
            for h in range(NH):
                ht, hr = h // 2, (h % 2) * HD
                P_tiles = []
                for jt in range(JT):
                    psp = psA.tile([128, N], F32, tag="pp", name="pp")
                    for ic in range(IC):
                        nc.tensor.matmul(
                            psp[:, 512 * ic:512 * (ic + 1)],
                            lhsT=mm(k_sb[ht][hr:hr + HD, 128 * jt:128 * (jt + 1)]),
                            rhs=mm(q_sb[ht][hr:hr + HD, 512 * ic:512 * (ic + 1)]),
                            start=True, stop=True)
                    pt = ppool.tile([128, N], F32, tag="P", name="P")
                    nc.scalar.activation(out=mm(pt[:]), in_=psp[:], func=AF.Exp)
                    P_tiles.append(pt)

                for ic in range(IC):
                    pav = psB.tile([128, 512], F32, tag="ps", name="ps")
                    for jt in range(JT):
                        nc.tensor.matmul(
                            pav[0:HD + 1, :],
                            lhsT=mm(vT_sb[jt][:, h, :]),
                            rhs=mm(P_tiles[jt][:, 512 * ic:512 * (ic + 1)]),
                            start=(jt == 0), stop=(jt == JT - 1))
                    # 1/Z on DVE, partition-broadcast via a DRAM bounce
                    # (SBUF APs cannot have stride-0 partitions, DRAM APs can)
                    rz = zrp.tile([1, 512], F32, tag="rz", name="rz")
                    nc.vector.reciprocal(out=rz[:], in_=pav[HD:HD + 1, :])
                    zd = zdp.tile([1, 512], F32, tag="zd", name="zd")
                    nc.sync.dma_start(out=zd[:], in_=rz[:])
                    zb = zbp.tile([HD, 512], F32, tag="zb", name="zb")
                    nc.sync.dma_start(out=zb[:],
                                      in_=zd[0, :].partition_broadcast(HD))
                    nc.vector.tensor_tensor(
                        out=mm(ao_sb[ht][hr:hr + HD, 512 * ic:512 * (ic + 1)]),
                        in0=pav[0:HD, :], in1=zb[:], op=AluOpType.mult)

            # ---- projection + bias + residual --------------------------
            for ot in range(CT):
                y = big.tile([128, N], F32, tag=f"k{ot}", name=f"y{ot}")  # reuse k
                for ic in range(IC):
                    psj = psB.tile([128, 512], F32, tag="ps", name="ps")
                    for ct in range(CT):
                        nc.tensor.matmul(
                            psj[:],
                            lhsT=mm(pw_sb[ct][:, 128 * ot:128 * (ot + 1)]),
                            rhs=mm(ao_sb[ct][:, 512 * ic:512 * (ic + 1)]),
                            start=(ct == 0), stop=(ct == CT - 1))
                    nc.vector.scalar_tensor_tensor(
                        out=y[:, 512 * ic:512 * (ic + 1)],
                        in0=psj[:], scalar=pb_sb[:, ot:ot + 1],
                        in1=x_sb[ot][:, 512 * ic:512 * (ic + 1)],
                        op0=AluOpType.add, op1=AluOpType.add)
                nc.sync.dma_start(out=io["out"][128 * ot:128 * (ot + 1), :], in_=y[:])


_NC_CACHE = {}


def _get_nc(mm_dtype=F32R, p_bufs=10):
    key = (str(mm_dtype), p_bufs)
    if key not in _NC_CACHE:
        nc = bacc.Bacc("TRN2", target_bir_lowering=False)
        io = _declare_io(nc)
        _build(nc, io, mm_dtype=mm_dtype, p_bufs=p_bufs)
        nc.compile()
        _NC_CACHE[key] = nc
    return _NC_CACHE[key]


def run(inputs, trace=False, **spmd_kwargs):
    """Build+run; returns (full_output, BassKernelResults)."""
    weights, xs = _host_prep(**inputs)
    nc = _get_nc()
    in_maps = [dict(weights, x=xs[b]) for b in range(NCORES)]
    res = run_bass_kernel_spmd(nc, in_maps, list(range(NCORES)),
                               trace=trace, **spmd_kwargs)
    out = np.stack([res.results[b]["out"].reshape(C, 32, 32)
                    for b in range(NCORES)]).astype(np.float32)
    return out, res


def kernel(**inputs):
    out, _ = run(inputs, trace=False)
    return out


if __name__ == "__main__":
    rng = np.random.default_rng(0)
    demo = {
        "x": rng.standard_normal((8, 512, 32, 32), dtype=np.float32),
        "gn_gamma": np.ones(512, np.float32),
        "gn_beta": np.zeros(512, np.float32),
        "qkv_w": rng.standard_normal((1536, 512), dtype=np.float32) / 22.6,
        "qkv_b": rng.standard_normal(1536, dtype=np.float32) * 0.02,
        "proj_w": rng.standard_normal((512, 512), dtype=np.float32) / 22.6,
        "proj_b": rng.standard_normal(512, dtype=np.float32) * 0.02,
    }
    print(kernel(**demo).shape)

